# revision 1
# baseline (speedup 1.0000x reference)
"""Trainium2 Bass kernel for a Neural CDE (fixed-step RK4 over a cubic spline).

Strategy
--------
Pure data-parallel over batch: 4096 samples -> 8 NeuronCores x 512.
Per core, activations live feature-major in SBUF: [C=128 partitions, B free].
The batch slice is split into NSUB sub-batches ("chains") pipelined against
each other -- each RK4 step is a serial chain of engine visits, so wall clock
~ n_steps * chain_latency; extra chains keep engines busy inside the latency.

Math notes
----------
- RK4 k_i are pre-scaled by their Butcher weight (k1' = dt/6 k1, k2' = dt/3 k2,
  k3' = dt/3 k3, k4' = dt/6 k4) so z_{n+1} = z_n + k1'+k2'+k3'+k4' is a plain
  sum, accumulated onto a persistent PSUM bank via identity matmuls.  The W1
  matmuls feeding the RK4 sub-states use rescaled weight copies (3W1, 1.5W1).
- Spline derivative planes dX(s) = c1 + 2s c2 + 3s^2 c3 are built per piece on
  s in {0,1/8,..,7/8} (+ s=1 on the final piece), pre-scaled by dt/6 (integer
  grid) or dt/3 (half grid), so the k-drain multiply needs no extra scaling.
- ELU exactly, with no native table function:
      elu(x) = max(x, min(exp(x), 1) - 1)
  one ACT Exp pass, one cheap clamp, one fused scalar_tensor_tensor max.
  exp overflow to inf still yields the correct branch.
- All constants + z0 ship in two packed DMAs (fp32 + fp16) so early matmuls
  depend on at most one DMA semaphore lane (PE instructions have a single
  HW sync-wait slot).
"""

import os
import sys

sys.path.insert(0, "/opt/trn_rl_repo")

import numpy as np

import concourse.bass as bass
import concourse.bacc as bacc
import concourse.mybir as mybir
import concourse.tile as tile
from concourse.bass_utils import run_bass_kernel_spmd

N_CORES = 8
B, P, C, H, O = 4096, 64, 128, 128, 10
BC = B // N_CORES  # 512 samples per core
SPP = 4  # RK4 steps per spline piece
DT = 1.0 / SPP
W6 = DT / 6.0  # weight for k1, k4
W3f = DT / 3.0  # weight for k2, k3

F32 = mybir.dt.float32
F16 = mybir.dt.float16
AL = mybir.AluOpType
AF = mybir.ActivationFunctionType

NSUB = int(os.environ.get("CDE_NSUB", "2"))

# fp32 pack layout (free-dim offsets): z0 | ident32 | w1 | wr | b1 b2 b3 br
_O_Z0 = 0
_O_I32 = _O_Z0 + BC
_O_W1 = _O_I32 + C
_O_WR = _O_W1 + H
_O_B1 = _O_WR + O
_O_B2 = _O_B1 + 1
_O_B3 = _O_B2 + 1
_O_BR = _O_B3 + 1
P32_TOT = _O_BR + 1
# fp16 pack layout: w1_3 | w1_15 | w2 | w3 | ident
P16_TOT = 5 * C


def build_kernel(n_pieces: int = P, nsub: int = NSUB) -> bass.Bass:
    fd = BC // nsub

    nc = bacc.Bacc("TRN2")

    pack32d = nc.dram_tensor("pack32", [C, P32_TOT], F32, kind="ExternalInput")
    pack16d = nc.dram_tensor("pack16", [C, P16_TOT], F16, kind="ExternalInput")
    cf = nc.dram_tensor("cf", [n_pieces, C, 3, BC], F32, kind="ExternalInput")
    outf = nc.dram_tensor("outf", [O, BC], F32, kind="ExternalOutput")

    with tile.TileContext(nc) as tc:
        with tc.tile_pool(name="const", bufs=1) as const:
            pk32 = const.tile([C, P32_TOT], F32)
            pk16 = const.tile([C, P16_TOT], F16)
            nc.sync.dma_start(pk32[:], pack32d[:])
            nc.sync.dma_start(pk16[:], pack16d[:])

            z0_sl = pk32[:, _O_Z0:_O_Z0 + BC]
            ident32 = pk32[:, _O_I32:_O_I32 + C]
            w1 = pk32[:, _O_W1:_O_W1 + H]
            wr = pk32[:, _O_WR:_O_WR + O]
            b1 = pk32[:, _O_B1:_O_B1 + 1]
            b2 = pk32[:, _O_B2:_O_B2 + 1]
            b3 = pk32[:, _O_B3:_O_B3 + 1]
            br = pk32[0:O, _O_BR:_O_BR + 1]
            w1_3 = pk16[:, 0 * C:1 * C]
            w1_15 = pk16[:, 1 * C:2 * C]
            w2 = pk16[:, 2 * C:3 * C]
            w3 = pk16[:, 3 * C:4 * C]
            ident = pk16[:, 4 * C:5 * C]

            _kernel_body(nc, tc, n_pieces, nsub, fd, z0_sl, cf, outf,
                         w1, w1_3, w1_15, w2, w3, ident, ident32, wr,
                         b1, b2, b3, br)
    nc.finalize()
    return nc


def _kernel_body(nc, tc, n_pieces, nsub, fd, z0_sl, cf, outf,
                 w1, w1_3, w1_15, w2, w3, ident, ident32, wr, b1, b2, b3, br):
    import contextlib
    ctx = contextlib.ExitStack()
    with ctx:
        coefp = ctx.enter_context(tc.tile_pool(name="coef", bufs=3))
        planep = ctx.enter_context(tc.tile_pool(name="plane", bufs=2))
        scratchp = ctx.enter_context(tc.tile_pool(name="scratch", bufs=4))
        zp = ctx.enter_context(tc.tile_pool(name="zsb", bufs=3))
        hp = ctx.enter_context(tc.tile_pool(name="hwork", bufs=3))
        kp = ctx.enter_context(tc.tile_pool(name="kwork", bufs=3))
        outp = ctx.enter_context(tc.tile_pool(name="outw", bufs=1))
        ps1 = ctx.enter_context(tc.tile_pool(name="ps1", bufs=2, space="PSUM"))
        ps2 = ctx.enter_context(tc.tile_pool(name="ps2", bufs=2, space="PSUM"))
        ps3 = ctx.enter_context(tc.tile_pool(name="ps3", bufs=2, space="PSUM"))
        psz = ctx.enter_context(tc.tile_pool(name="psz", bufs=1, space="PSUM"))
        psout = ctx.enter_context(tc.tile_pool(name="psout", bufs=1,
                                               space="PSUM"))

        # persistent Z accumulator (PSUM, fp32), seeded with z0 via identity
        # matmul (sets has_written so later start=False matmuls accumulate)
        zacc = psz.tile([C, BC], F32, name="zacc")
        nc.tensor.matmul(zacc[:], ident32, z0_sl, start=True, stop=False,
                         skip_group_check=True)
        z_sb = z0_sl  # current z, feature-major [C, BC] fp32 (SBUF)

        coef_tiles = {}
        plane_tiles = {}

        def load_piece(p):
            ct = coefp.tile([C, 3 * BC], F16, name=f"coef_{p}", tag="coef")
            nc.gpsimd.dma_start(ct[:], cf[p])  # f32 -> f16 cast DMA
            coef_tiles[p] = ct

        def build_planes(p):
            ct = coef_tiles[p]
            c1 = ct[:, 0 * BC:1 * BC]
            c2 = ct[:, 1 * BC:2 * BC]
            c3 = ct[:, 2 * BC:3 * BC]
            pl = planep.tile([C, 8 * BC], F16, name=f"plane_{p}", tag="plane")
            plane_tiles[p] = pl
            nc.vector.tensor_scalar(pl[:, 0:BC], c1, W6, None, AL.mult)
            c1w3 = scratchp.tile([C, BC], F16, name=f"c1w3_{p}", tag="c1w3")
            nc.vector.tensor_scalar(c1w3[:], c1, W3f, None, AL.mult)
            for sl in range(1, 8):
                s = sl / 8.0
                w = W6 if sl % 2 == 0 else W3f
                base = pl[:, 0:BC] if sl % 2 == 0 else c1w3[:]
                u = scratchp.tile([C, BC], F16, name=f"u_{p}_{sl}",
                                  tag="uplane")
                nc.vector.scalar_tensor_tensor(
                    u[:], c2, 2.0 * s * w, base, AL.mult, AL.add)
                nc.vector.scalar_tensor_tensor(
                    pl[:, sl * BC:(sl + 1) * BC], c3, 3.0 * s * s * w, u[:],
                    AL.mult, AL.add)

        def build_plane_s1(p):
            ct = coef_tiles[p]
            c2 = ct[:, 1 * BC:2 * BC]
            c3 = ct[:, 2 * BC:3 * BC]
            pl1 = scratchp.tile([C, BC], F16, name="plane_s1", tag="plane_s1")
            u = scratchp.tile([C, BC], F16, name="u_s1", tag="uplane")
            nc.vector.scalar_tensor_tensor(
                u[:], c2, 2.0 * W6, plane_tiles[p][:, 0:BC], AL.mult, AL.add)
            nc.vector.scalar_tensor_tensor(
                pl1[:], c3, 3.0 * W6, u[:], AL.mult, AL.add)
            return pl1

        load_piece(0)
        build_planes(0)
        if n_pieces > 1:
            load_piece(1)
            build_planes(1)
        extra_s1 = None

        def sub(t, s):
            return t[:, s * fd:(s + 1) * fd]

        def mlp_tail(e_psum_ap, plane_ap, relu_on_act):
            """ELU -> L2 -> ReLU -> L3 -> k' drain for one eval/sub-batch."""
            e = hp.tile([C, fd], F16, name="e_exp", tag="e_exp")
            nc.scalar.activation(e[:], e_psum_ap, AF.Exp, bias=b1, scale=1.0)
            t = hp.tile([C, fd], F16, name="t_clamp", tag="t_clamp")
            nc.gpsimd.tensor_scalar(t[:], e[:], 1.0, -1.0, AL.min, AL.add)
            h1 = hp.tile([C, fd], F16, name="h1", tag="h1")
            nc.vector.scalar_tensor_tensor(
                h1[:], e_psum_ap, b1, t[:], AL.add, AL.max)

            a2 = ps2.tile([H, fd], F32, name="a2", tag="a2")
            nc.tensor.matmul(a2[:], w2, h1[:], start=True, stop=True)
            h2 = hp.tile([H, fd], F16, name="h2", tag="h2")
            if relu_on_act:
                nc.scalar.activation(h2[:], a2[:], AF.Relu, bias=b2, scale=1.0)
            else:
                nc.vector.tensor_scalar(h2[:], a2[:], b2, 0.0, AL.add, AL.max)

            a3 = ps3.tile([C, fd], F32, name="a3", tag="a3")
            nc.tensor.matmul(a3[:], w3, h2[:], start=True, stop=True)
            k = kp.tile([C, fd], F16, name="kdrain", tag="kdrain")
            nc.vector.scalar_tensor_tensor(
                k[:], a3[:], b3, plane_ap, AL.add, AL.mult)
            return k

        # ================= main time loop =================
        for p in range(n_pieces):
            if p + 2 < n_pieces:
                load_piece(p + 2)
            if p + 1 < n_pieces and (p + 1) not in plane_tiles:
                build_planes(p + 1)
            if p == n_pieces - 1:
                extra_s1 = build_plane_s1(p)
            pl = plane_tiles[p]
            pl_next = plane_tiles.get(p + 1)

            for j in range(SPP):
                sa = pl[:, (2 * j) * BC:(2 * j + 1) * BC]
                sb_ = pl[:, (2 * j + 1) * BC:(2 * j + 2) * BC]
                if j < SPP - 1:
                    sc = pl[:, (2 * j + 2) * BC:(2 * j + 3) * BC]
                elif p + 1 < n_pieces:
                    sc = pl_next[:, 0:BC]
                else:
                    sc = extra_s1[:]

                z_new = zp.tile([C, BC], F32, name=f"z_{p}_{j}", tag="znew")
                last_step = (p == n_pieces - 1 and j == SPP - 1)
                for s in range(nsub):
                    fsl = slice(s * fd, (s + 1) * fd)
                    e1 = ps1.tile([H, fd], F32, name="e1", tag="e1")
                    nc.tensor.matmul(e1[:], w1, sub(z_sb, s),
                                     start=True, stop=True)
                    k1 = mlp_tail(e1[:], sa[:, fsl], relu_on_act=True)

                    e2 = ps1.tile([H, fd], F32, name="e2", tag="e1")
                    nc.tensor.matmul(e2[:], w1, sub(z_sb, s),
                                     start=True, stop=False)
                    nc.tensor.matmul(e2[:], w1_3, k1[:],
                                     start=False, stop=True)
                    k2 = mlp_tail(e2[:], sb_[:, fsl], relu_on_act=False)

                    e3 = ps1.tile([H, fd], F32, name="e3", tag="e1")
                    nc.tensor.matmul(e3[:], w1, sub(z_sb, s),
                                     start=True, stop=False)
                    nc.tensor.matmul(e3[:], w1_15, k2[:],
                                     start=False, stop=True)
                    k3 = mlp_tail(e3[:], sb_[:, fsl], relu_on_act=True)

                    e4 = ps1.tile([H, fd], F32, name="e4", tag="e1")
                    nc.tensor.matmul(e4[:], w1, sub(z_sb, s),
                                     start=True, stop=False)
                    nc.tensor.matmul(e4[:], w1_3, k3[:],
                                     start=False, stop=True)
                    k4 = mlp_tail(e4[:], sc[:, fsl], relu_on_act=False)

                    zs = zacc[:, fsl]
                    for ki, kt in enumerate((k1, k2, k3, k4)):
                        nc.tensor.matmul(
                            zs, ident, kt[:],
                            start=False,
                            stop=(last_step and ki == 3),
                            skip_group_check=True,
                        )
                    nc.scalar.copy(z_new[:, fsl], zs)
                z_sb = z_new[:]

        op = psout.tile([O, BC], F32, name="ops")
        nc.tensor.matmul(op[:], wr, z_sb, start=True, stop=True)
        out_sb = outp.tile([O, BC], F32, name="out_sb")
        nc.scalar.activation(out_sb[:], op[:], AF.Identity, bias=br, scale=1.0)
        nc.sync.dma_start(outf[:], out_sb[:])


# ---------------------------------------------------------------------------
# host side
# ---------------------------------------------------------------------------

_BUILT = {}


def _get_kernel(n_pieces=P, nsub=NSUB):
    key = (n_pieces, nsub)
    if key not in _BUILT:
        _BUILT[key] = build_kernel(n_pieces, nsub)
    return _BUILT[key]


def _prep_inputs(z0, coeffs, W1, b1, W2, b2, W3, b3, Wr, br, n_pieces=P):
    z0 = np.asarray(z0, np.float32)
    coeffs = np.asarray(coeffs, np.float32)
    W1 = np.asarray(W1, np.float32)

    z0c = z0.reshape(N_CORES, BC, C).transpose(0, 2, 1)  # [core, C, BC]
    cc = coeffs[:, :n_pieces, :, 1:4]  # [B, P, C, 3]
    cc = np.ascontiguousarray(
        cc.reshape(N_CORES, BC, n_pieces, C, 3).transpose(0, 2, 3, 4, 1))

    pack32 = np.zeros((N_CORES, C, P32_TOT), np.float32)
    pack32[:, :, _O_Z0:_O_Z0 + BC] = z0c
    pack32[:, :, _O_I32:_O_I32 + C] = np.eye(C, dtype=np.float32)
    pack32[:, :, _O_W1:_O_W1 + H] = W1
    pack32[:, :H, _O_WR:_O_WR + O] = np.asarray(Wr, np.float32)
    pack32[:, :H, _O_B1] = np.asarray(b1, np.float32)
    pack32[:, :H, _O_B2] = np.asarray(b2, np.float32)
    pack32[:, :C, _O_B3] = np.asarray(b3, np.float32)
    pack32[:, :O, _O_BR] = np.asarray(br, np.float32)

    pack16 = np.zeros((C, P16_TOT), np.float16)
    pack16[:, 0 * C:1 * C] = (3.0 * W1).astype(np.float16)
    pack16[:, 1 * C:2 * C] = (1.5 * W1).astype(np.float16)
    pack16[:, 2 * C:3 * C] = np.asarray(W2, np.float16)
    pack16[:, 3 * C:4 * C] = np.asarray(W3, np.float16)
    pack16[:, 4 * C:5 * C] = np.eye(C, dtype=np.float16)

    in_maps = []
    for c in range(N_CORES):
        in_maps.append({
            "pack32": np.ascontiguousarray(pack32[c]),
            "pack16": pack16,
            "cf": cc[c],
        })
    return in_maps


def run(z0, coeffs, W1, b1, W2, b2, W3, b3, Wr, br,
        n_pieces=P, nsub=NSUB, trace=False):
    nc = _get_kernel(n_pieces, nsub)
    in_maps = _prep_inputs(z0, coeffs, W1, b1, W2, b2, W3, b3, Wr, br,
                           n_pieces=n_pieces)
    res = run_bass_kernel_spmd(nc, in_maps, core_ids=list(range(N_CORES)),
                               trace=trace)
    outs = [res.results[c]["outf"] for c in range(N_CORES)]  # [O, BC]
    out = np.concatenate([o.T for o in outs], axis=0)  # [B, O]
    return np.asarray(out, np.float32), res


def kernel(z0, coeffs, W1, b1, W2, b2, W3, b3, Wr, br):
    out, _ = run(z0, coeffs, W1, b1, W2, b2, W3, b3, Wr, br)
    return out



# revision 6
# speedup vs baseline: 1.8104x; 1.8104x over previous
"""Trainium2 Bass kernel for a Neural CDE (fixed-step RK4 over a cubic spline).

Strategy (v3)
-------------
Pure data-parallel over batch: 4096 samples -> 8 NeuronCores x 512.
Per core, activations live feature-major in SBUF: [C=128 partitions, B free].
The 512-sample slice is split into NSUB chains (default 4 x 128) pipelined
against each other: each RK4 step is a serial engine chain, so wall clock
~ n_steps * chain_latency; the chains fill the engines inside that latency.

Everything is fp16 (validated 3.4e-3 rel err vs the fp32 reference on CPU):
z state, weights, planes, k tiles.  No PSUM accumulator state, no GpSimd
ops (its software tensor_scalar measured 4.6us each in the v1 trace), no
fp32 matmuls (4 cycles/row vs 1 for fp16).

Per eval: e = W1 @ zin (PE) -> ELU -> a2 = W2 @ h1 (PE) -> ReLU ->
a3 = W3' @ h2 (PE) -> k' = (a3 + b3') * plane (DVE) -> zin_next =
3|1.5 * k' + z (DVE).  Butcher weights (dt/6, dt/3) are folded into two
scaled copies of W3, so planes are the *raw* spline derivative on the
half-step grid (s = g/8) and the zin scalars are 3 / 1.5 / 3 exactly.

ELU has no native table; two decompositions, selectable per eval to
balance the Scalar(ACT) and Vector(DVE) engines:
  form A (ACT-heavy):  r = Relu(a1 + b1); u = Relu(1 - exp(a1 + b1));
                       h1 = r - u                       (2 ACT + 1 DVE)
  form B (DVE-heavy):  e = exp(a1 + b1); t = min(e,1) - 1;
                       h1 = max(a1 + b1, t)             (1 ACT + 2 DVE)
Both are exact (exp overflow to inf is absorbed by min/relu clamps).

z update: z' = ((z + (k1'+k2')) + (k3'+k4')) with the first add issued
mid-step (off the critical chain).
"""

import os
import sys

sys.path.insert(0, "/opt/trn_rl_repo")

import numpy as np

import concourse.bass as bass
import concourse.bacc as bacc
import concourse.mybir as mybir
import concourse.tile as tile
from concourse.bass_utils import run_bass_kernel_spmd

N_CORES = 8
B, P, C, H, O = 4096, 64, 128, 128, 10
BC = B // N_CORES  # 512 samples per core
SPP = 4  # RK4 steps per spline piece
DT = 1.0 / SPP

F32 = mybir.dt.float32
F16 = mybir.dt.float16
AL = mybir.AluOpType
AF = mybir.ActivationFunctionType

NSUB = int(os.environ.get("CDE_NSUB", "4"))
FORMS = os.environ.get("CDE_FORMS", "AABB")  # ELU form per RK4 eval
RELU2 = os.environ.get("CDE_RELU2", "act")  # 'act' | 'dve' (dve needs b2==0)

# fp16 pack layout (free-dim offsets): z0 | w1 | w2 | w3_6 | w3_3 | wr
_O_Z0 = 0
_O_W1 = _O_Z0 + BC
_O_W2 = _O_W1 + H
_O_W36 = _O_W2 + H
_O_W33 = _O_W36 + C
_O_WR = _O_W33 + C
P16_TOT = _O_WR + O
# fp32 pack layout: b1 | b2 | b3_6 | b3_3 | br
P32_TOT = 5


def build_kernel(n_pieces: int = P, nsub: int = NSUB, forms: str = FORMS,
                 relu2: str = RELU2) -> bass.Bass:
    fd = BC // nsub

    nc = bacc.Bacc("TRN2")

    pack16d = nc.dram_tensor("pack16", [C, P16_TOT], F16, kind="ExternalInput")
    pack32d = nc.dram_tensor("pack32", [C, P32_TOT], F32, kind="ExternalInput")
    cf = nc.dram_tensor("cf", [n_pieces, C, 3, BC], F32, kind="ExternalInput")
    outf = nc.dram_tensor("outf", [O, BC], F32, kind="ExternalOutput")

    with tile.TileContext(nc) as tc:
        with tc.tile_pool(name="const", bufs=1) as const:
            pk16 = const.tile([C, P16_TOT], F16)
            pk32 = const.tile([C, P32_TOT], F32)
            nc.sync.dma_start(pk16[:], pack16d[:])
            nc.sync.dma_start(pk32[:], pack32d[:])

            z0_sl = pk16[:, _O_Z0:_O_Z0 + BC]
            w1 = pk16[:, _O_W1:_O_W1 + H]
            w2 = pk16[:, _O_W2:_O_W2 + H]
            w36 = pk16[:, _O_W36:_O_W36 + C]
            w33 = pk16[:, _O_W33:_O_W33 + C]
            wr = pk16[:, _O_WR:_O_WR + O]
            b1 = pk32[:, 0:1]
            b2 = pk32[:, 1:2]
            b36 = pk32[:, 2:3]
            b33 = pk32[:, 3:4]
            br = pk32[0:O, 4:5]

            _kernel_body(nc, tc, n_pieces, nsub, fd, forms, relu2,
                         z0_sl, cf, outf, w1, w2, w36, w33, wr,
                         b1, b2, b36, b33, br)
    nc.finalize()
    return nc


def _kernel_body(nc, tc, n_pieces, nsub, fd, forms, relu2, z0_sl, cf, outf,
                 w1, w2, w36, w33, wr, b1, b2, b36, b33, br):
    import contextlib
    ctx = contextlib.ExitStack()
    with ctx:
        coefp = ctx.enter_context(tc.tile_pool(name="coef", bufs=3))
        planep = ctx.enter_context(tc.tile_pool(name="plane", bufs=2))
        zp = ctx.enter_context(tc.tile_pool(name="zsb", bufs=2))
        hp = ctx.enter_context(tc.tile_pool(name="hwork", bufs=2))
        kp = ctx.enter_context(tc.tile_pool(name="kwork", bufs=2))
        outp = ctx.enter_context(tc.tile_pool(name="outw", bufs=1))
        ps1 = ctx.enter_context(tc.tile_pool(name="ps1", bufs=2, space="PSUM"))
        ps2 = ctx.enter_context(tc.tile_pool(name="ps2", bufs=2, space="PSUM"))
        ps3 = ctx.enter_context(tc.tile_pool(name="ps3", bufs=2, space="PSUM"))
        psout = ctx.enter_context(tc.tile_pool(name="psout", bufs=1,
                                               space="PSUM"))

        coef_tiles = {}
        plane_tiles = {}

        def load_piece(p):
            ct = coefp.tile([C, 3 * BC], F16, name=f"coef_{p}", tag="coef")
            nc.gpsimd.dma_start(ct[:], cf[p])  # f32 -> f16 cast DMA
            coef_tiles[p] = ct

        def c_slice(p, i):  # i in 0..2 -> c1, c2, c3
            return coef_tiles[p][:, i * BC:(i + 1) * BC]

        scratchp = ctx.enter_context(tc.tile_pool(name="scratch", bufs=2))

        def plane_ops(p):
            """Emit list of thunks building planes for piece p.

            plane grid g in 1..7: dX(g/8) = c1 + (g/4) c2 + (3g^2/64) c3.
            Grid 0 is the raw c1 slice (no build).  Stored in a
            [C, 7*BC] tile, grid g at cols (g-1)*BC.
            """
            pl = planep.tile([C, 7 * BC], F16, name=f"plane_{p}", tag="plane")
            plane_tiles[p] = pl
            ops = []
            for g in range(1, 8):
                s = g / 8.0
                dst = pl[:, (g - 1) * BC:g * BC]
                box = {}

                def op1(box=box, p=p, s=s, g=g):
                    u = scratchp.tile([C, BC], F16, name=f"u_{p}_{g}",
                                      tag="uplane")
                    box["u"] = u
                    nc.vector.scalar_tensor_tensor(
                        u[:], c_slice(p, 1), 2.0 * s, c_slice(p, 0),
                        AL.mult, AL.add)

                def op2(box=box, dst=dst, p=p, s=s):
                    nc.vector.scalar_tensor_tensor(
                        dst, c_slice(p, 2), 3.0 * s * s, box["u"][:],
                        AL.mult, AL.add)
                ops += [op1, op2]
            return ops

        def grid_ap(p, g):
            if g == 0:
                return c_slice(p, 0)
            return plane_tiles[p][:, (g - 1) * BC:g * BC]

        def build_plane_s1(p):
            pl1 = planep.tile([C, BC], F16, name="plane_s1", tag="plane_s1")
            u = scratchp.tile([C, BC], F16, name="u_s1", tag="uplane")
            nc.vector.scalar_tensor_tensor(
                u[:], c_slice(p, 1), 2.0, c_slice(p, 0), AL.mult, AL.add)
            nc.vector.scalar_tensor_tensor(
                pl1[:], c_slice(p, 2), 3.0, u[:], AL.mult, AL.add)
            return pl1

        # prologue: piece 0 (+1) coeffs and piece 0 planes fully built
        load_piece(0)
        if n_pieces > 1:
            load_piece(1)
        for op in plane_ops(0):
            op()
        pending_plane_ops = plane_ops(1) if n_pieces > 1 else []
        extra_s1 = None

        z_sb = z0_sl  # current z, [C, BC] fp16 SBUF

        def fsl(s):
            return slice(s * fd, (s + 1) * fd)

        # ================= main time loop =================
        for p in range(n_pieces):
            for j in range(SPP):
                step = p * SPP + j
                last_step = (p == n_pieces - 1 and j == SPP - 1)

                # background work: next-piece planes + coeff prefetch
                if j == 0 and p + 2 < n_pieces:
                    load_piece(p + 2)
                if p == n_pieces - 1 and j == 0:
                    extra_s1 = build_plane_s1(p)
                if pending_plane_ops:
                    chunk = 4 if j < 2 else 3
                    for op in pending_plane_ops[:chunk]:
                        op()
                    del pending_plane_ops[:chunk]
                if j == SPP - 1 and p + 1 < n_pieces - 1:
                    pending_plane_ops = plane_ops(p + 1 + 1)
                elif j == SPP - 1 and p + 1 == n_pieces - 1:
                    pending_plane_ops = plane_ops(p + 1)
                    # last piece: build eagerly enough (7*2 ops over 4 steps)

                # plane refs for this step's 4 evals
                g1 = grid_ap(p, 2 * j)
                g23 = grid_ap(p, 2 * j + 1)
                if j < SPP - 1:
                    g4 = grid_ap(p, 2 * j + 2)
                elif p + 1 < n_pieces:
                    g4 = grid_ap(p + 1, 0)
                else:
                    g4 = extra_s1[:]

                z_new = zp.tile([C, BC], F16, name=f"z_{step}", tag="z")
                kt = [kp.tile([C, BC], F16, name=f"k{i}_{step}", tag=f"k{i}")
                      for i in range(4)]
                t12 = kp.tile([C, BC], F16, name=f"t12_{step}", tag="t12")
                t34 = kp.tile([C, BC], F16, name=f"t34_{step}", tag="t34")
                za = kp.tile([C, BC], F16, name=f"za_{step}", tag="za")

                zin = z_sb
                for ev in range(4):
                    w3x = w36 if ev in (0, 3) else w33
                    b3x = b36 if ev in (0, 3) else b33
                    gpl = (g1, g23, g23, g4)[ev]
                    form = forms[ev]

                    a1 = ps1.tile([H, BC], F32, name=f"a1_{step}_{ev}",
                                  tag="a1")
                    a2 = ps2.tile([H, BC], F32, name=f"a2_{step}_{ev}",
                                  tag="a2")
                    a3 = ps3.tile([C, BC], F32, name=f"a3_{step}_{ev}",
                                  tag="a3")
                    h1 = hp.tile([C, BC], F16, name=f"h1_{step}_{ev}",
                                 tag="h1")
                    h2 = hp.tile([C, BC], F16, name=f"h2_{step}_{ev}",
                                 tag="h2")

                    # L1 matmuls (PE), per chain
                    for s in range(nsub):
                        nc.tensor.matmul(a1[:, fsl(s)], w1, zin[:, fsl(s)],
                                         start=True, stop=True)
                    # ELU
                    if form == "A":
                        eb = hp.tile([C, BC], F16, name=f"e_{step}_{ev}",
                                     tag="e")
                        rb = hp.tile([C, BC], F16, name=f"r_{step}_{ev}",
                                     tag="r")
                        ub = hp.tile([C, BC], F16, name=f"u_{step}_{ev}",
                                     tag="u")
                        for s in range(nsub):
                            sl = fsl(s)
                            nc.scalar.activation(eb[:, sl], a1[:, sl], AF.Exp,
                                                 bias=b1, scale=1.0)
                            nc.scalar.activation(ub[:, sl], eb[:, sl], AF.Relu,
                                                 bias=1.0, scale=-1.0)
                            nc.scalar.activation(rb[:, sl], a1[:, sl], AF.Relu,
                                                 bias=b1, scale=1.0)
                        for s in range(nsub):
                            sl = fsl(s)
                            nc.vector.tensor_tensor(h1[:, sl], rb[:, sl],
                                                    ub[:, sl], AL.subtract)
                    else:  # form B
                        eb = hp.tile([C, BC], F16, name=f"e_{step}_{ev}",
                                     tag="e")
                        tb = hp.tile([C, BC], F16, name=f"t_{step}_{ev}",
                                     tag="t")
                        for s in range(nsub):
                            sl = fsl(s)
                            nc.scalar.activation(eb[:, sl], a1[:, sl], AF.Exp,
                                                 bias=b1, scale=1.0)
                        for s in range(nsub):
                            sl = fsl(s)
                            nc.vector.tensor_scalar(tb[:, sl], eb[:, sl],
                                                    1.0, -1.0, AL.min, AL.add)
                            nc.vector.scalar_tensor_tensor(
                                h1[:, sl], a1[:, sl], b1, tb[:, sl],
                                AL.add, AL.max)

                    # L2 matmul + ReLU
                    for s in range(nsub):
                        nc.tensor.matmul(a2[:, fsl(s)], w2, h1[:, fsl(s)],
                                         start=True, stop=True)
                    for s in range(nsub):
                        sl = fsl(s)
                        if relu2 == "act":
                            nc.scalar.activation(h2[:, sl], a2[:, sl], AF.Relu,
                                                 bias=b2, scale=1.0)
                        else:
                            nc.vector.tensor_scalar(h2[:, sl], a2[:, sl],
                                                    0.0, None, AL.max)

                    # L3 matmul + k-drain (+ zin for next eval)
                    for s in range(nsub):
                        nc.tensor.matmul(a3[:, fsl(s)], w3x, h2[:, fsl(s)],
                                         start=True, stop=True)
                    zin_next = (None if ev == 3 else
                                kp.tile([C, BC], F16, name=f"zin_{step}_{ev}",
                                        tag=f"zin{ev}"))
                    zin_scale = (3.0, 1.5, 3.0, 0.0)[ev]
                    for s in range(nsub):
                        sl = fsl(s)
                        nc.vector.scalar_tensor_tensor(
                            kt[ev][:, sl], a3[:, sl], b3x, gpl[:, sl],
                            AL.add, AL.mult)
                        if ev < 3:
                            nc.vector.scalar_tensor_tensor(
                                zin_next[:, sl], kt[ev][:, sl], zin_scale,
                                z_sb[:, sl], AL.mult, AL.add)
                    # mid-step z partials (off critical chain)
                    if ev == 1:
                        for s in range(nsub):
                            sl = fsl(s)
                            nc.vector.tensor_tensor(t12[:, sl], kt[0][:, sl],
                                                    kt[1][:, sl], AL.add)
                    if ev == 2:
                        for s in range(nsub):
                            sl = fsl(s)
                            nc.vector.tensor_tensor(za[:, sl], z_sb[:, sl],
                                                    t12[:, sl], AL.add)
                    if ev == 3:
                        for s in range(nsub):
                            sl = fsl(s)
                            nc.vector.tensor_tensor(t34[:, sl], kt[2][:, sl],
                                                    kt[3][:, sl], AL.add)
                            nc.vector.tensor_tensor(z_new[:, sl], za[:, sl],
                                                    t34[:, sl], AL.add)
                    if ev < 3:
                        zin = zin_next
                z_sb = z_new[:]

        op = psout.tile([O, BC], F32, name="ops")
        nc.tensor.matmul(op[:], wr, z_sb, start=True, stop=True)
        out_sb = outp.tile([O, BC], F32, name="out_sb")
        nc.scalar.activation(out_sb[:], op[:], AF.Identity, bias=br, scale=1.0)
        nc.sync.dma_start(outf[:], out_sb[:])


# ---------------------------------------------------------------------------
# host side
# ---------------------------------------------------------------------------

_BUILT = {}


def _get_kernel(n_pieces=P, nsub=NSUB, forms=FORMS, relu2=RELU2):
    key = (n_pieces, nsub, forms, relu2)
    if key not in _BUILT:
        _BUILT[key] = build_kernel(n_pieces, nsub, forms, relu2)
    return _BUILT[key]


def _prep_inputs(z0, coeffs, W1, b1, W2, b2, W3, b3, Wr, br, n_pieces=P):
    z0 = np.asarray(z0, np.float32)
    coeffs = np.asarray(coeffs, np.float32)

    z0c = z0.reshape(N_CORES, BC, C).transpose(0, 2, 1)  # [core, C, BC]
    cc = coeffs[:, :n_pieces, :, 1:4]  # [B, P, C, 3]
    cc = np.ascontiguousarray(
        cc.reshape(N_CORES, BC, n_pieces, C, 3).transpose(0, 2, 3, 4, 1))

    pack16 = np.zeros((N_CORES, C, P16_TOT), np.float16)
    pack16[:, :, _O_Z0:_O_Z0 + BC] = z0c.astype(np.float16)
    pack16[:, :, _O_W1:_O_W1 + H] = np.asarray(W1, np.float16)
    pack16[:, :, _O_W2:_O_W2 + H] = np.asarray(W2, np.float16)
    pack16[:, :, _O_W36:_O_W36 + C] = (np.asarray(W3, np.float32)
                                       * (DT / 6.0)).astype(np.float16)
    pack16[:, :, _O_W33:_O_W33 + C] = (np.asarray(W3, np.float32)
                                       * (DT / 3.0)).astype(np.float16)
    pack16[:, :H, _O_WR:_O_WR + O] = np.asarray(Wr, np.float16)

    pack32 = np.zeros((C, P32_TOT), np.float32)
    pack32[:H, 0] = np.asarray(b1, np.float32)
    pack32[:H, 1] = np.asarray(b2, np.float32)
    pack32[:C, 2] = np.asarray(b3, np.float32) * (DT / 6.0)
    pack32[:C, 3] = np.asarray(b3, np.float32) * (DT / 3.0)
    pack32[:O, 4] = np.asarray(br, np.float32)

    in_maps = []
    for c in range(N_CORES):
        in_maps.append({
            "pack16": np.ascontiguousarray(pack16[c]),
            "pack32": pack32,
            "cf": cc[c],
        })
    return in_maps


def run(z0, coeffs, W1, b1, W2, b2, W3, b3, Wr, br,
        n_pieces=P, nsub=NSUB, forms=FORMS, relu2=RELU2, trace=False):
    nc = _get_kernel(n_pieces, nsub, forms, relu2)
    in_maps = _prep_inputs(z0, coeffs, W1, b1, W2, b2, W3, b3, Wr, br,
                           n_pieces=n_pieces)
    res = run_bass_kernel_spmd(nc, in_maps, core_ids=list(range(N_CORES)),
                               trace=trace)
    outs = [res.results[c]["outf"] for c in range(N_CORES)]  # [O, BC]
    out = np.concatenate([o.T for o in outs], axis=0)  # [B, O]
    return np.asarray(out, np.float32), res


def kernel(z0, coeffs, W1, b1, W2, b2, W3, b3, Wr, br):
    out, _ = run(z0, coeffs, W1, b1, W2, b2, W3, b3, Wr, br)
    return out


# revision 20
# speedup vs baseline: 3.2164x; 1.7767x over previous
"""Trainium2 Bass kernel for a Neural CDE (fixed-step RK4 over a cubic spline).

Strategy (v3)
-------------
Pure data-parallel over batch: 4096 samples -> 8 NeuronCores x 512.
Per core, activations live feature-major in SBUF: [C=128 partitions, B free].
The 512-sample slice is split into NSUB chains (default 4 x 128) pipelined
against each other: each RK4 step is a serial engine chain, so wall clock
~ n_steps * chain_latency; the chains fill the engines inside that latency.

Everything is fp16 (validated 3.4e-3 rel err vs the fp32 reference on CPU):
z state, weights, planes, k tiles.  No PSUM accumulator state, no GpSimd
ops (its software tensor_scalar measured 4.6us each in the v1 trace), no
fp32 matmuls (4 cycles/row vs 1 for fp16).

Per eval: e = W1 @ zin (PE) -> ELU -> a2 = W2 @ h1 (PE) -> ReLU ->
a3 = W3' @ h2 (PE) -> k' = (a3 + b3') * plane (DVE) -> zin_next =
3|1.5 * k' + z (DVE).  Butcher weights (dt/6, dt/3) are folded into two
scaled copies of W3, so planes are the *raw* spline derivative on the
half-step grid (s = g/8) and the zin scalars are 3 / 1.5 / 3 exactly.

ELU has no native table; two decompositions, selectable per eval to
balance the Scalar(ACT) and Vector(DVE) engines:
  form A (ACT-heavy):  r = Relu(a1 + b1); u = Relu(1 - exp(a1 + b1));
                       h1 = r - u                       (2 ACT + 1 DVE)
  form B (DVE-heavy):  e = exp(a1 + b1); t = min(e,1) - 1;
                       h1 = max(a1 + b1, t)             (1 ACT + 2 DVE)
Both are exact (exp overflow to inf is absorbed by min/relu clamps).

z update: z' = ((z + (k1'+k2')) + (k3'+k4')) with the first add issued
mid-step (off the critical chain).
"""

import os
import sys

sys.path.insert(0, "/opt/trn_rl_repo")

import numpy as np

import concourse.bass as bass
import concourse.bacc as bacc
import concourse.mybir as mybir
import concourse.tile as tile
from concourse.bass_utils import run_bass_kernel_spmd

N_CORES = 8
B, P, C, H, O = 4096, 64, 128, 128, 10
BC = B // N_CORES  # 512 samples per core
SPP = 4  # RK4 steps per spline piece
DT = 1.0 / SPP

F32 = mybir.dt.float32
F16 = mybir.dt.float16
AL = mybir.AluOpType
AF = mybir.ActivationFunctionType

NSUB = int(os.environ.get("CDE_NSUB", "4"))
FORMS = os.environ.get("CDE_FORMS", "AABB")  # ELU form per RK4 eval
RELU2 = os.environ.get("CDE_RELU2", "act")  # 'act' | 'dve' (dve needs b2==0)
ZIN = os.environ.get("CDE_ZIN", "dve")  # 'dve' stt | 'pe' accumulate-matmul

# fp16 pack layout (free-dim offsets): z0 | w1 | w2 | w3_6 | w3_3 | wr
_O_Z0 = 0
_O_W1 = _O_Z0 + BC
_O_W2 = _O_W1 + H
_O_W36 = _O_W2 + H
_O_W33 = _O_W36 + C
_O_WR = _O_W33 + C
_O_W13 = _O_WR + O   # 3*W1 (zin 'pe' route)
_O_W115 = _O_W13 + H  # 1.5*W1
P16_TOT = _O_W115 + H
# fp32 pack layout: b1 | b2 | b3_6 | b3_3 | br
P32_TOT = 5


def build_kernel(n_pieces: int = P, nsub: int = NSUB, forms: str = FORMS,
                 relu2: str = RELU2, zin_route: str = ZIN) -> bass.Bass:
    fd = BC // nsub

    nc = bacc.Bacc("TRN2")

    pack16d = nc.dram_tensor("pack16", [C, P16_TOT], F16, kind="ExternalInput")
    pack32d = nc.dram_tensor("pack32", [C, P32_TOT], F32, kind="ExternalInput")
    cf = nc.dram_tensor("cf", [n_pieces, C, 3, BC], F32, kind="ExternalInput")
    outf = nc.dram_tensor("outf", [O, BC], F32, kind="ExternalOutput")

    with tile.TileContext(nc) as tc:
        with tc.tile_pool(name="const", bufs=1) as const:
            pk16 = const.tile([C, P16_TOT], F16)
            pk32 = const.tile([C, P32_TOT], F32)
            nc.sync.dma_start(pk16[:], pack16d[:])
            nc.sync.dma_start(pk32[:], pack32d[:])

            z0_sl = pk16[:, _O_Z0:_O_Z0 + BC]
            w1 = pk16[:, _O_W1:_O_W1 + H]
            w2 = pk16[:, _O_W2:_O_W2 + H]
            w36 = pk16[:, _O_W36:_O_W36 + C]
            w33 = pk16[:, _O_W33:_O_W33 + C]
            wr = pk16[:, _O_WR:_O_WR + O]
            w13 = pk16[:, _O_W13:_O_W13 + H]
            w115 = pk16[:, _O_W115:_O_W115 + H]
            b1 = pk32[:, 0:1]
            b2 = pk32[:, 1:2]
            b36 = pk32[:, 2:3]
            b33 = pk32[:, 3:4]
            br = pk32[0:O, 4:5]

            _kernel_body(nc, tc, n_pieces, nsub, fd, forms, relu2, zin_route,
                         z0_sl, cf, outf, w1, w2, w36, w33, wr, w13, w115,
                         b1, b2, b36, b33, br)
    nc.finalize()
    return nc


def _kernel_body(nc, tc, n_pieces, nsub, fd, forms, relu2, zin_route,
                 z0_sl, cf, outf, w1, w2, w36, w33, wr, w13, w115,
                 b1, b2, b36, b33, br):
    import contextlib
    ctx = contextlib.ExitStack()
    with ctx:
        coefp = ctx.enter_context(tc.tile_pool(name="coef", bufs=3))
        planep = ctx.enter_context(tc.tile_pool(name="plane", bufs=2))
        zp = ctx.enter_context(tc.tile_pool(name="zsb", bufs=2))
        hp = ctx.enter_context(tc.tile_pool(name="hwork", bufs=2))
        kp = ctx.enter_context(tc.tile_pool(name="kwork", bufs=2))
        outp = ctx.enter_context(tc.tile_pool(name="outw", bufs=1))
        # one PSUM bank per chain, reused across the 3 MLP stages of an
        # eval (each stage's matmul overwrites it only after the previous
        # stage's reader is done -- which the serial chain guarantees)
        pa = ctx.enter_context(tc.tile_pool(name="pa", bufs=1, space="PSUM"))
        psout = ctx.enter_context(tc.tile_pool(name="psout", bufs=1,
                                               space="PSUM"))

        coef_tiles = {}
        plane_tiles = {}

        def load_piece(p):
            ct = coefp.tile([C, 3 * BC], F16, name=f"coef_{p}", tag="coef")
            nc.gpsimd.dma_start(ct[:], cf[p])  # f32 -> f16 cast DMA
            coef_tiles[p] = ct

        def c_slice(p, i):  # i in 0..2 -> c1, c2, c3
            return coef_tiles[p][:, i * BC:(i + 1) * BC]

        scratchp = ctx.enter_context(tc.tile_pool(name="scratch", bufs=2))

        def plane_ops(p):
            """Emit list of thunks building planes for piece p.

            plane grid g in 1..7: dX(g/8) = c1 + (g/4) c2 + (3g^2/64) c3.
            Grid 0 is the raw c1 slice (no build).  Stored in a
            [C, 7*BC] tile, grid g at cols (g-1)*BC.
            """
            pl = planep.tile([C, 7 * BC], F16, name=f"plane_{p}", tag="plane")
            plane_tiles[p] = pl
            ops = []
            for g in range(1, 8):
                s = g / 8.0
                dst = pl[:, (g - 1) * BC:g * BC]
                box = {}

                def op1(box=box, p=p, s=s, g=g):
                    u = scratchp.tile([C, BC], F16, name=f"u_{p}_{g}",
                                      tag="uplane")
                    box["u"] = u
                    nc.vector.scalar_tensor_tensor(
                        u[:], c_slice(p, 1), 2.0 * s, c_slice(p, 0),
                        AL.mult, AL.add)

                def op2(box=box, dst=dst, p=p, s=s):
                    nc.vector.scalar_tensor_tensor(
                        dst, c_slice(p, 2), 3.0 * s * s, box["u"][:],
                        AL.mult, AL.add)
                ops += [op1, op2]
            return ops

        def grid_ap(p, g):
            if g == 0:
                return c_slice(p, 0)
            return plane_tiles[p][:, (g - 1) * BC:g * BC]

        def build_plane_s1(p):
            pl1 = planep.tile([C, BC], F16, name="plane_s1", tag="plane_s1")
            u = scratchp.tile([C, BC], F16, name="u_s1", tag="uplane")
            nc.vector.scalar_tensor_tensor(
                u[:], c_slice(p, 1), 2.0, c_slice(p, 0), AL.mult, AL.add)
            nc.vector.scalar_tensor_tensor(
                pl1[:], c_slice(p, 2), 3.0, u[:], AL.mult, AL.add)
            return pl1

        # prologue: piece 0 (+1) coeffs and piece 0 planes fully built
        load_piece(0)
        if n_pieces > 1:
            load_piece(1)
        for op in plane_ops(0):
            op()
        pending_plane_ops = []
        extra_s1 = None

        # current z per chain, [C, fd] fp16 SBUF
        z_sb = [z0_sl[:, c * fd:(c + 1) * fd] for c in range(nsub)]

        def psl(ap, c):  # slice a full-BC plane/coef column range for chain c
            return ap[:, c * fd:(c + 1) * fd]

        # ================= main time loop =================
        for p in range(n_pieces):
            for j in range(SPP):
                step = p * SPP + j

                # coeff prefetch (gpsimd DMA queue, idle engine)
                if j == 0 and p + 2 < n_pieces:
                    load_piece(p + 2)
                if p == n_pieces - 1 and j == 0:
                    extra_s1 = build_plane_s1(p)
                if j == 0 and p + 1 < n_pieces:
                    assert not pending_plane_ops
                    pending_plane_ops = plane_ops(p + 1)

                # plane refs for this step's 4 evals
                g1 = grid_ap(p, 2 * j)
                g23 = grid_ap(p, 2 * j + 1)
                if j < SPP - 1:
                    g4 = grid_ap(p, 2 * j + 2)
                elif p + 1 < n_pieces:
                    g4 = grid_ap(p + 1, 0)
                else:
                    g4 = extra_s1[:]

                z_new = [zp.tile([C, fd], F16, name=f"z_{step}_{c}",
                                 tag=f"z{c}") for c in range(nsub)]
                kt = [[kp.tile([C, fd], F16, name=f"k{i}_{step}_{c}",
                               tag=f"k{i}_{c}") for c in range(nsub)]
                      for i in range(4)]
                t12 = [kp.tile([C, fd], F16, name=f"t12_{step}_{c}",
                               tag=f"t12_{c}") for c in range(nsub)]
                t34 = [kp.tile([C, fd], F16, name=f"t34_{step}_{c}",
                               tag=f"t34_{c}") for c in range(nsub)]
                za = [kp.tile([C, fd], F16, name=f"za_{step}_{c}",
                              tag=f"za_{c}") for c in range(nsub)]

                zin = z_sb
                for ev in range(4):
                    w3x = w36 if ev in (0, 3) else w33
                    b3x = b36 if ev in (0, 3) else b33
                    gpl = (g1, g23, g23, g4)[ev]
                    form = forms[ev]

                    a1 = [pa.tile([H, fd], F32, name=f"a1_{step}_{ev}_{c}",
                                  tag=f"a{c}") for c in range(nsub)]
                    h1 = [hp.tile([C, fd], F16, name=f"h1_{step}_{ev}_{c}",
                                  tag=f"h1_{c}") for c in range(nsub)]

                    for c in range(nsub):
                        if zin_route == "pe" and ev > 0:
                            # e = W1 @ z + sc*W1 @ k_prev  (two accumulating
                            # matmuls; zin never materialized)
                            w1x = w115 if ev == 2 else w13
                            nc.tensor.matmul(a1[c][:], w1, z_sb[c][:],
                                             start=True, stop=False)
                            nc.tensor.matmul(a1[c][:], w1x, kt[ev - 1][c][:],
                                             start=False, stop=True)
                        else:
                            nc.tensor.matmul(a1[c][:], w1, zin[c][:],
                                             start=True, stop=True)
                    # ELU
                    if form == "A":
                        for c in range(nsub):
                            eb = hp.tile([C, fd], F16, name=f"e_{step}_{ev}_{c}",
                                         tag=f"e_{c}")
                            rb = hp.tile([C, fd], F16, name=f"r_{step}_{ev}_{c}",
                                         tag=f"r_{c}")
                            nc.scalar.activation(eb[:], a1[c][:], AF.Exp,
                                                 bias=b1, scale=1.0)
                            nc.scalar.activation(rb[:], a1[c][:], AF.Relu,
                                                 bias=b1, scale=1.0)
                            ub = hp.tile([C, fd], F16, name=f"u_{step}_{ev}_{c}",
                                         tag=f"u_{c}")
                            nc.scalar.activation(ub[:], eb[:], AF.Relu,
                                                 bias=1.0, scale=-1.0)
                            nc.vector.tensor_tensor(h1[c][:], rb[:], ub[:],
                                                    AL.subtract)
                    else:  # form B
                        for c in range(nsub):
                            eb = hp.tile([C, fd], F16, name=f"e_{step}_{ev}_{c}",
                                         tag=f"e_{c}")
                            tb = hp.tile([C, fd], F16, name=f"t_{step}_{ev}_{c}",
                                         tag=f"t_{c}")
                            nc.scalar.activation(eb[:], a1[c][:], AF.Exp,
                                                 bias=b1, scale=1.0)
                            nc.vector.tensor_scalar(tb[:], eb[:],
                                                    1.0, -1.0, AL.min, AL.add)
                            nc.vector.scalar_tensor_tensor(
                                h1[c][:], a1[c][:], b1, tb[:],
                                AL.add, AL.max)

                    # L2 matmul + ReLU (a-bank reused: write waits h1 read)
                    a2 = [pa.tile([H, fd], F32, name=f"a2_{step}_{ev}_{c}",
                                  tag=f"a{c}") for c in range(nsub)]
                    h2 = [hp.tile([C, fd], F16, name=f"h2_{step}_{ev}_{c}",
                                  tag=f"h2_{c}") for c in range(nsub)]
                    for c in range(nsub):
                        nc.tensor.matmul(a2[c][:], w2, h1[c][:],
                                         start=True, stop=True)
                        if relu2 == "act":
                            nc.scalar.activation(h2[c][:], a2[c][:], AF.Relu,
                                                 bias=b2, scale=1.0)
                        else:
                            nc.vector.tensor_scalar(h2[c][:], a2[c][:],
                                                    0.0, None, AL.max)

                    # L3 matmul + k-drain (+ zin for next eval)
                    a3 = [pa.tile([C, fd], F32, name=f"a3_{step}_{ev}_{c}",
                                  tag=f"a{c}") for c in range(nsub)]
                    want_zin = (ev < 3 and zin_route != "pe")
                    zin_next = ([kp.tile([C, fd], F16,
                                         name=f"zin_{step}_{ev}_{c}",
                                         tag=f"zin{ev}_{c}")
                                 for c in range(nsub)] if want_zin else None)
                    zin_scale = (3.0, 1.5, 3.0, 0.0)[ev]
                    for c in range(nsub):
                        nc.tensor.matmul(a3[c][:], w3x, h2[c][:],
                                         start=True, stop=True)
                    for c in range(nsub):
                        nc.vector.scalar_tensor_tensor(
                            kt[ev][c][:], a3[c][:], b3x, psl(gpl, c),
                            AL.add, AL.mult)
                        if want_zin:
                            nc.vector.scalar_tensor_tensor(
                                zin_next[c][:], kt[ev][c][:], zin_scale,
                                z_sb[c][:], AL.mult, AL.add)
                    # mid-step z partials (off critical chain)
                    if ev == 1:
                        for c in range(nsub):
                            nc.vector.tensor_tensor(t12[c][:], kt[0][c][:],
                                                    kt[1][c][:], AL.add)
                        # plane build for an upcoming piece: emit in the
                        # middle of the step so it never heads the DVE queue
                        if pending_plane_ops:
                            chunk = 4 if j < 2 else 3
                            for op in pending_plane_ops[:chunk]:
                                op()
                            del pending_plane_ops[:chunk]
                    if ev == 2:
                        for c in range(nsub):
                            nc.vector.tensor_tensor(za[c][:], z_sb[c][:],
                                                    t12[c][:], AL.add)
                    if ev == 3:
                        for c in range(nsub):
                            nc.vector.tensor_tensor(t34[c][:], kt[2][c][:],
                                                    kt[3][c][:], AL.add)
                            nc.vector.tensor_tensor(z_new[c][:], za[c][:],
                                                    t34[c][:], AL.add)
                    if want_zin:
                        zin = zin_next
                z_sb = z_new

        op = psout.tile([O, BC], F32, name="ops")
        for c in range(nsub):
            nc.tensor.matmul(op[:, c * fd:(c + 1) * fd], wr, z_sb[c][:],
                             start=True, stop=True)
        out_sb = outp.tile([O, BC], F32, name="out_sb")
        nc.scalar.activation(out_sb[:], op[:], AF.Identity, bias=br, scale=1.0)
        nc.sync.dma_start(outf[:], out_sb[:])


# ---------------------------------------------------------------------------
# host side
# ---------------------------------------------------------------------------

_BUILT = {}


def _get_kernel(n_pieces=P, nsub=NSUB, forms=FORMS, relu2=RELU2, zin=ZIN):
    key = (n_pieces, nsub, forms, relu2, zin)
    if key not in _BUILT:
        _BUILT[key] = build_kernel(n_pieces, nsub, forms, relu2, zin)
    return _BUILT[key]


def _prep_inputs(z0, coeffs, W1, b1, W2, b2, W3, b3, Wr, br, n_pieces=P):
    z0 = np.asarray(z0, np.float32)
    coeffs = np.asarray(coeffs, np.float32)

    z0c = z0.reshape(N_CORES, BC, C).transpose(0, 2, 1)  # [core, C, BC]
    cc = coeffs[:, :n_pieces, :, 1:4]  # [B, P, C, 3]
    cc = np.ascontiguousarray(
        cc.reshape(N_CORES, BC, n_pieces, C, 3).transpose(0, 2, 3, 4, 1))

    pack16 = np.zeros((N_CORES, C, P16_TOT), np.float16)
    pack16[:, :, _O_Z0:_O_Z0 + BC] = z0c.astype(np.float16)
    pack16[:, :, _O_W1:_O_W1 + H] = np.asarray(W1, np.float16)
    pack16[:, :, _O_W2:_O_W2 + H] = np.asarray(W2, np.float16)
    pack16[:, :, _O_W36:_O_W36 + C] = (np.asarray(W3, np.float32)
                                       * (DT / 6.0)).astype(np.float16)
    pack16[:, :, _O_W33:_O_W33 + C] = (np.asarray(W3, np.float32)
                                       * (DT / 3.0)).astype(np.float16)
    pack16[:, :H, _O_WR:_O_WR + O] = np.asarray(Wr, np.float16)
    pack16[:, :, _O_W13:_O_W13 + H] = (3.0 * np.asarray(W1, np.float32)
                                       ).astype(np.float16)
    pack16[:, :, _O_W115:_O_W115 + H] = (1.5 * np.asarray(W1, np.float32)
                                         ).astype(np.float16)

    pack32 = np.zeros((C, P32_TOT), np.float32)
    pack32[:H, 0] = np.asarray(b1, np.float32)
    pack32[:H, 1] = np.asarray(b2, np.float32)
    pack32[:C, 2] = np.asarray(b3, np.float32) * (DT / 6.0)
    pack32[:C, 3] = np.asarray(b3, np.float32) * (DT / 3.0)
    pack32[:O, 4] = np.asarray(br, np.float32)

    in_maps = []
    for c in range(N_CORES):
        in_maps.append({
            "pack16": np.ascontiguousarray(pack16[c]),
            "pack32": pack32,
            "cf": cc[c],
        })
    return in_maps


def run(z0, coeffs, W1, b1, W2, b2, W3, b3, Wr, br,
        n_pieces=P, nsub=NSUB, forms=FORMS, relu2=RELU2, zin=ZIN,
        trace=False):
    nc = _get_kernel(n_pieces, nsub, forms, relu2, zin)
    in_maps = _prep_inputs(z0, coeffs, W1, b1, W2, b2, W3, b3, Wr, br,
                           n_pieces=n_pieces)
    res = run_bass_kernel_spmd(nc, in_maps, core_ids=list(range(N_CORES)),
                               trace=trace)
    outs = [res.results[c]["outf"] for c in range(N_CORES)]  # [O, BC]
    out = np.concatenate([o.T for o in outs], axis=0)  # [B, O]
    return np.asarray(out, np.float32), res


def kernel(z0, coeffs, W1, b1, W2, b2, W3, b3, Wr, br):
    out, _ = run(z0, coeffs, W1, b1, W2, b2, W3, b3, Wr, br)
    return out


# revision 33
# speedup vs baseline: 3.8078x; 1.1838x over previous
"""Trainium2 Bass kernel for a Neural CDE (fixed-step RK4 over a cubic spline).

Strategy (v3)
-------------
Pure data-parallel over batch: 4096 samples -> 8 NeuronCores x 512.
Per core, activations live feature-major in SBUF: [C=128 partitions, B free].
The 512-sample slice is split into NSUB chains (default 4 x 128) pipelined
against each other: each RK4 step is a serial engine chain, so wall clock
~ n_steps * chain_latency; the chains fill the engines inside that latency.

Everything is fp16 (validated 3.4e-3 rel err vs the fp32 reference on CPU):
z state, weights, planes, k tiles.  No PSUM accumulator state, no GpSimd
ops (its software tensor_scalar measured 4.6us each in the v1 trace), no
fp32 matmuls (4 cycles/row vs 1 for fp16).

Per eval: e = W1 @ zin (PE) -> ELU -> a2 = W2 @ h1 (PE) -> ReLU ->
a3 = W3' @ h2 (PE) -> k' = (a3 + b3') * plane (DVE) -> zin_next =
3|1.5 * k' + z (DVE).  Butcher weights (dt/6, dt/3) are folded into two
scaled copies of W3, so planes are the *raw* spline derivative on the
half-step grid (s = g/8) and the zin scalars are 3 / 1.5 / 3 exactly.

ELU has no native table; two decompositions, selectable per eval to
balance the Scalar(ACT) and Vector(DVE) engines:
  form A (ACT-heavy):  r = Relu(a1 + b1); u = Relu(1 - exp(a1 + b1));
                       h1 = r - u                       (2 ACT + 1 DVE)
  form B (DVE-heavy):  e = exp(a1 + b1); t = min(e,1) - 1;
                       h1 = max(a1 + b1, t)             (1 ACT + 2 DVE)
Both are exact (exp overflow to inf is absorbed by min/relu clamps).

z update: z' = ((z + (k1'+k2')) + (k3'+k4')) with the first add issued
mid-step (off the critical chain).
"""

import os
import sys

sys.path.insert(0, "/opt/trn_rl_repo")

import numpy as np

import concourse.bass as bass
import concourse.bacc as bacc
import concourse.mybir as mybir
import concourse.tile as tile
from concourse.bass_utils import run_bass_kernel_spmd

N_CORES = 8
B, P, C, H, O = 4096, 64, 128, 128, 10
BC = B // N_CORES  # 512 samples per core
SPP = 4  # RK4 steps per spline piece
DT = 1.0 / SPP

F32 = mybir.dt.float32
F16 = mybir.dt.float16
AL = mybir.AluOpType
AF = mybir.ActivationFunctionType

NSUB = int(os.environ.get("CDE_NSUB", "4"))
FORMS = os.environ.get("CDE_FORMS", "AABB")  # ELU form per RK4 eval
RELU2 = os.environ.get("CDE_RELU2", "dddd")  # per eval: 'a' ACT | 'd' DVE ts
ZIN = os.environ.get("CDE_ZIN", "pe")  # 'dve' stt | 'pe' accumulate-matmul

# fp16 pack layout (free-dim offsets): z0 | w1 | w2 | w3_6 | w3_3 | wr
_O_Z0 = 0
_O_W1 = _O_Z0 + BC
_O_W2 = _O_W1 + H
_O_W36 = _O_W2 + H
_O_W33 = _O_W36 + C
_O_WR = _O_W33 + C
_O_W13 = _O_WR + O   # 3*W1 (zin 'pe' route)
_O_W115 = _O_W13 + H  # 1.5*W1
P16_TOT = _O_W115 + H
# fp32 pack layout: b1 | b2 | b3_6 | b3_3 | br
P32_TOT = 5


def build_kernel(n_pieces: int = P, nsub: int = NSUB, forms: str = FORMS,
                 relu2: str = RELU2, zin_route: str = ZIN,
                 b1z: bool = True, b2z: bool = True,
                 b3z: bool = True) -> bass.Bass:
    fd = BC // nsub
    if not b2z:
        relu2 = "aaaa"

    nc = bacc.Bacc("TRN2")

    pack16d = nc.dram_tensor("pack16", [C, P16_TOT], F16, kind="ExternalInput")
    pack32d = nc.dram_tensor("pack32", [C, P32_TOT], F32, kind="ExternalInput")
    # host-precomputed spline-derivative planes: grid g at s=g/8 per piece,
    # plus the s=1 plane of the final piece (pl1)
    pld = nc.dram_tensor("pl", [n_pieces, C, 8, BC], F16, kind="ExternalInput")
    pl1d = nc.dram_tensor("pl1", [C, BC], F16, kind="ExternalInput")
    outf = nc.dram_tensor("outf", [O, BC], F32, kind="ExternalOutput")

    with tile.TileContext(nc) as tc:
        with tc.tile_pool(name="const", bufs=1) as const:
            pk16 = const.tile([C, P16_TOT], F16)
            pk32 = const.tile([C, P32_TOT], F32)
            nc.sync.dma_start(pk16[:], pack16d[:])
            nc.sync.dma_start(pk32[:], pack32d[:])

            z0_sl = pk16[:, _O_Z0:_O_Z0 + BC]
            w1 = pk16[:, _O_W1:_O_W1 + H]
            w2 = pk16[:, _O_W2:_O_W2 + H]
            w36 = pk16[:, _O_W36:_O_W36 + C]
            w33 = pk16[:, _O_W33:_O_W33 + C]
            wr = pk16[:, _O_WR:_O_WR + O]
            w13 = pk16[:, _O_W13:_O_W13 + H]
            w115 = pk16[:, _O_W115:_O_W115 + H]
            b1 = pk32[:, 0:1]
            b2 = pk32[:, 1:2]
            b36 = pk32[:, 2:3]
            b33 = pk32[:, 3:4]
            br = pk32[0:O, 4:5]

            _kernel_body(nc, tc, n_pieces, nsub, fd, forms, relu2, zin_route,
                         b1z, b3z, z0_sl, pld, pl1d, outf,
                         w1, w2, w36, w33, wr, w13, w115,
                         b1, b2, b36, b33, br)
    nc.finalize()
    return nc


def _kernel_body(nc, tc, n_pieces, nsub, fd, forms, relu2, zin_route,
                 b1z, b3z, z0_sl, pld, pl1d, outf,
                 w1, w2, w36, w33, wr, w13, w115,
                 b1, b2, b36, b33, br):
    import contextlib
    ctx = contextlib.ExitStack()
    with ctx:
        planep = ctx.enter_context(tc.tile_pool(name="plane", bufs=3))
        zp = ctx.enter_context(tc.tile_pool(name="zsb", bufs=2))
        hp = ctx.enter_context(tc.tile_pool(name="hwork", bufs=2))
        kp = ctx.enter_context(tc.tile_pool(name="kwork", bufs=2))
        outp = ctx.enter_context(tc.tile_pool(name="outw", bufs=1))
        # one PSUM bank per chain, reused across the 3 MLP stages of an
        # eval (each stage's matmul overwrites it only after the previous
        # stage's reader is done -- which the serial chain guarantees)
        pa = ctx.enter_context(tc.tile_pool(name="pa", bufs=1, space="PSUM"))
        psout = ctx.enter_context(tc.tile_pool(name="psout", bufs=1,
                                               space="PSUM"))

        plane_tiles = {}

        def load_piece(p):
            pl = planep.tile([C, 8 * BC], F16, name=f"plane_{p}", tag="plane")
            nc.gpsimd.dma_start(pl[:], pld[p])
            plane_tiles[p] = pl

        def grid_ap(p, g):
            return plane_tiles[p][:, g * BC:(g + 1) * BC]

        # prologue: first two pieces' planes in flight + the final s=1 plane
        load_piece(0)
        if n_pieces > 1:
            load_piece(1)
        extra_s1 = planep.tile([C, BC], F16, name="plane_s1", tag="plane_s1")
        nc.gpsimd.dma_start(extra_s1[:], pl1d[:])

        # current z per chain, [C, fd] fp16 SBUF
        z_sb = [z0_sl[:, c * fd:(c + 1) * fd] for c in range(nsub)]

        def psl(ap, c):  # slice a full-BC plane/coef column range for chain c
            return ap[:, c * fd:(c + 1) * fd]

        # ================= main time loop =================
        for p in range(n_pieces):
            for j in range(SPP):
                step = p * SPP + j

                # plane prefetch (gpsimd DMA queue, idle engine)
                if j == 0 and p + 2 < n_pieces:
                    load_piece(p + 2)

                # plane refs for this step's 4 evals
                g1 = grid_ap(p, 2 * j)
                g23 = grid_ap(p, 2 * j + 1)
                if j < SPP - 1:
                    g4 = grid_ap(p, 2 * j + 2)
                elif p + 1 < n_pieces:
                    g4 = grid_ap(p + 1, 0)
                else:
                    g4 = extra_s1[:]

                z_new = [zp.tile([C, fd], F16, name=f"z_{step}_{c}",
                                 tag=f"z{c}") for c in range(nsub)]
                kt = [[kp.tile([C, fd], F16, name=f"k{i}_{step}_{c}",
                               tag=f"k{i}_{c}") for c in range(nsub)]
                      for i in range(4)]
                t12 = [kp.tile([C, fd], F16, name=f"t12_{step}_{c}",
                               tag=f"t12_{c}") for c in range(nsub)]
                t34 = [kp.tile([C, fd], F16, name=f"t34_{step}_{c}",
                               tag=f"t34_{c}") for c in range(nsub)]
                za = [kp.tile([C, fd], F16, name=f"za_{step}_{c}",
                              tag=f"za_{c}") for c in range(nsub)]

                zin = z_sb
                for ev in range(4):
                    w3x = w36 if ev in (0, 3) else w33
                    b3x = b36 if ev in (0, 3) else b33
                    gpl = (g1, g23, g23, g4)[ev]
                    form = forms[ev]

                    a1 = [pa.tile([H, fd], F32, name=f"a1_{step}_{ev}_{c}",
                                  tag=f"a{c}") for c in range(nsub)]
                    h1 = [hp.tile([C, fd], F16, name=f"h1_{step}_{ev}_{c}",
                                  tag=f"h1_{c}") for c in range(nsub)]

                    for c in range(nsub):
                        if zin_route == "pe" and ev > 0:
                            # e = W1 @ z + sc*W1 @ k_prev  (two accumulating
                            # matmuls; zin never materialized)
                            w1x = w115 if ev == 2 else w13
                            nc.tensor.matmul(a1[c][:], w1, z_sb[c][:],
                                             start=True, stop=False)
                            nc.tensor.matmul(a1[c][:], w1x, kt[ev - 1][c][:],
                                             start=False, stop=True)
                        else:
                            nc.tensor.matmul(a1[c][:], w1, zin[c][:],
                                             start=True, stop=True)
                    # ELU
                    if form == "A":
                        for c in range(nsub):
                            eb = hp.tile([C, fd], F16, name=f"e_{step}_{ev}_{c}",
                                         tag=f"e_{c}")
                            rb = hp.tile([C, fd], F16, name=f"r_{step}_{ev}_{c}",
                                         tag=f"r_{c}")
                            nc.scalar.activation(eb[:], a1[c][:], AF.Exp,
                                                 bias=b1, scale=1.0)
                            nc.scalar.activation(rb[:], a1[c][:], AF.Relu,
                                                 bias=b1, scale=1.0)
                            ub = hp.tile([C, fd], F16, name=f"u_{step}_{ev}_{c}",
                                         tag=f"u_{c}")
                            nc.scalar.activation(ub[:], eb[:], AF.Relu,
                                                 bias=1.0, scale=-1.0)
                            nc.vector.tensor_tensor(h1[c][:], rb[:], ub[:],
                                                    AL.subtract)
                    else:  # form B
                        for c in range(nsub):
                            eb = hp.tile([C, fd], F16, name=f"e_{step}_{ev}_{c}",
                                         tag=f"e_{c}")
                            tb = hp.tile([C, fd], F16, name=f"t_{step}_{ev}_{c}",
                                         tag=f"t_{c}")
                            nc.scalar.activation(eb[:], a1[c][:], AF.Exp,
                                                 bias=b1, scale=1.0)
                            nc.vector.tensor_scalar(tb[:], eb[:],
                                                    1.0, -1.0, AL.min, AL.add)
                            if b1z:
                                nc.vector.tensor_tensor(
                                    h1[c][:], a1[c][:], tb[:], AL.max)
                            else:
                                nc.vector.scalar_tensor_tensor(
                                    h1[c][:], a1[c][:], b1, tb[:],
                                    AL.add, AL.max)

                    # L2 matmul + ReLU (a-bank reused: write waits h1 read)
                    a2 = [pa.tile([H, fd], F32, name=f"a2_{step}_{ev}_{c}",
                                  tag=f"a{c}") for c in range(nsub)]
                    h2 = [hp.tile([C, fd], F16, name=f"h2_{step}_{ev}_{c}",
                                  tag=f"h2_{c}") for c in range(nsub)]
                    for c in range(nsub):
                        nc.tensor.matmul(a2[c][:], w2, h1[c][:],
                                         start=True, stop=True)
                        if relu2[ev] == "a":
                            nc.scalar.activation(h2[c][:], a2[c][:], AF.Relu,
                                                 bias=b2, scale=1.0)
                        else:
                            nc.vector.tensor_scalar(h2[c][:], a2[c][:],
                                                    0.0, None, AL.max)

                    # L3 matmul + k-drain (+ zin for next eval)
                    a3 = [pa.tile([C, fd], F32, name=f"a3_{step}_{ev}_{c}",
                                  tag=f"a{c}") for c in range(nsub)]
                    want_zin = (ev < 3 and zin_route != "pe")
                    zin_next = ([kp.tile([C, fd], F16,
                                         name=f"zin_{step}_{ev}_{c}",
                                         tag=f"zin{ev}_{c}")
                                 for c in range(nsub)] if want_zin else None)
                    zin_scale = (3.0, 1.5, 3.0, 0.0)[ev]
                    for c in range(nsub):
                        nc.tensor.matmul(a3[c][:], w3x, h2[c][:],
                                         start=True, stop=True)
                    for c in range(nsub):
                        if b3z:
                            nc.vector.tensor_tensor(
                                kt[ev][c][:], a3[c][:], psl(gpl, c), AL.mult)
                        else:
                            nc.vector.scalar_tensor_tensor(
                                kt[ev][c][:], a3[c][:], b3x, psl(gpl, c),
                                AL.add, AL.mult)
                        if want_zin:
                            nc.vector.scalar_tensor_tensor(
                                zin_next[c][:], kt[ev][c][:], zin_scale,
                                z_sb[c][:], AL.mult, AL.add)
                    # mid-step z partials (off critical chain)
                    if ev == 1:
                        for c in range(nsub):
                            nc.vector.tensor_tensor(t12[c][:], kt[0][c][:],
                                                    kt[1][c][:], AL.add)
                    if ev == 2:
                        for c in range(nsub):
                            nc.vector.tensor_tensor(za[c][:], z_sb[c][:],
                                                    t12[c][:], AL.add)
                    if ev == 3:
                        for c in range(nsub):
                            nc.vector.tensor_tensor(t34[c][:], kt[2][c][:],
                                                    kt[3][c][:], AL.add)
                            nc.vector.tensor_tensor(z_new[c][:], za[c][:],
                                                    t34[c][:], AL.add)
                    if want_zin:
                        zin = zin_next
                z_sb = z_new

        op = psout.tile([O, BC], F32, name="ops")
        for c in range(nsub):
            nc.tensor.matmul(op[:, c * fd:(c + 1) * fd], wr, z_sb[c][:],
                             start=True, stop=True)
        out_sb = outp.tile([O, BC], F32, name="out_sb")
        nc.scalar.activation(out_sb[:], op[:], AF.Identity, bias=br, scale=1.0)
        nc.sync.dma_start(outf[:], out_sb[:])


# ---------------------------------------------------------------------------
# host side
# ---------------------------------------------------------------------------

_BUILT = {}


def _get_kernel(n_pieces=P, nsub=NSUB, forms=FORMS, relu2=RELU2, zin=ZIN,
                b1z=True, b2z=True, b3z=True):
    key = (n_pieces, nsub, forms, relu2, zin, b1z, b2z, b3z)
    if key not in _BUILT:
        _BUILT[key] = build_kernel(n_pieces, nsub, forms, relu2, zin,
                                   b1z, b2z, b3z)
    return _BUILT[key]


def _prep_inputs(z0, coeffs, W1, b1, W2, b2, W3, b3, Wr, br, n_pieces=P):
    z0 = np.asarray(z0, np.float32)
    coeffs = np.asarray(coeffs, np.float32)

    z0c = z0.reshape(N_CORES, BC, C).transpose(0, 2, 1)  # [core, C, BC]

    # spline-derivative planes dX(s) = c1 + 2s c2 + 3s^2 c3 on s = g/8,
    # fp16, laid out [core, P, C, 8, BC]
    cc = coeffs[:, :n_pieces, :, 1:4].reshape(N_CORES, BC, n_pieces, C, 3)
    cc = cc.transpose(0, 2, 3, 4, 1)  # [core, P, C, 3, BC]
    s = np.arange(8, dtype=np.float32) / 8.0
    planes = (cc[:, :, :, None, 0, :]
              + (2.0 * s)[None, None, None, :, None] * cc[:, :, :, None, 1, :]
              + (3.0 * s * s)[None, None, None, :, None]
              * cc[:, :, :, None, 2, :]).astype(np.float16)
    pl1 = (cc[:, n_pieces - 1, :, 0] + 2.0 * cc[:, n_pieces - 1, :, 1]
           + 3.0 * cc[:, n_pieces - 1, :, 2]).astype(np.float16)  # s=1

    pack16 = np.zeros((N_CORES, C, P16_TOT), np.float16)
    pack16[:, :, _O_Z0:_O_Z0 + BC] = z0c.astype(np.float16)
    pack16[:, :, _O_W1:_O_W1 + H] = np.asarray(W1, np.float16)
    pack16[:, :, _O_W2:_O_W2 + H] = np.asarray(W2, np.float16)
    pack16[:, :, _O_W36:_O_W36 + C] = (np.asarray(W3, np.float32)
                                       * (DT / 6.0)).astype(np.float16)
    pack16[:, :, _O_W33:_O_W33 + C] = (np.asarray(W3, np.float32)
                                       * (DT / 3.0)).astype(np.float16)
    pack16[:, :H, _O_WR:_O_WR + O] = np.asarray(Wr, np.float16)
    pack16[:, :, _O_W13:_O_W13 + H] = (3.0 * np.asarray(W1, np.float32)
                                       ).astype(np.float16)
    pack16[:, :, _O_W115:_O_W115 + H] = (1.5 * np.asarray(W1, np.float32)
                                         ).astype(np.float16)

    pack32 = np.zeros((C, P32_TOT), np.float32)
    pack32[:H, 0] = np.asarray(b1, np.float32)
    pack32[:H, 1] = np.asarray(b2, np.float32)
    pack32[:C, 2] = np.asarray(b3, np.float32) * (DT / 6.0)
    pack32[:C, 3] = np.asarray(b3, np.float32) * (DT / 3.0)
    pack32[:O, 4] = np.asarray(br, np.float32)

    in_maps = []
    for c in range(N_CORES):
        in_maps.append({
            "pack16": np.ascontiguousarray(pack16[c]),
            "pack32": pack32,
            "pl": np.ascontiguousarray(planes[c]),
            "pl1": np.ascontiguousarray(pl1[c]),
        })
    return in_maps


def run(z0, coeffs, W1, b1, W2, b2, W3, b3, Wr, br,
        n_pieces=P, nsub=NSUB, forms=FORMS, relu2=RELU2, zin=ZIN,
        trace=False):
    b1z = bool(np.all(np.asarray(b1) == 0.0))
    b2z = bool(np.all(np.asarray(b2) == 0.0))
    b3z = bool(np.all(np.asarray(b3) == 0.0))
    nc = _get_kernel(n_pieces, nsub, forms, relu2, zin, b1z, b2z, b3z)
    in_maps = _prep_inputs(z0, coeffs, W1, b1, W2, b2, W3, b3, Wr, br,
                           n_pieces=n_pieces)
    res = run_bass_kernel_spmd(nc, in_maps, core_ids=list(range(N_CORES)),
                               trace=trace)
    outs = [res.results[c]["outf"] for c in range(N_CORES)]  # [O, BC]
    out = np.concatenate([o.T for o in outs], axis=0)  # [B, O]
    return np.asarray(out, np.float32), res


def kernel(z0, coeffs, W1, b1, W2, b2, W3, b3, Wr, br):
    out, _ = run(z0, coeffs, W1, b1, W2, b2, W3, b3, Wr, br)
    return out
